# revision 3
# baseline (speedup 1.0000x reference)
"""Trainium2 Bass kernel for nn_DeterministicEncoder — binned prefix-table algorithm.

  out[n] = e^{-q_n} A(q_n) + e^{q_n} (T_B - B(q_n)),
  A(q) = sum_{k_m <= q} e^{k_m} v_m,  B(q) = sum_{k_m <= q} e^{-k_m} v_m.

A/B are staircases in q; approximate on a C=128-boundary grid g_b (validated
rel err ~5e-3 vs the 2e-2 gate):
  PATS[c, b]  = sum_m tab[m, c] * mask_b(k_m)        (64 mask matmuls)
  D[b]        = (PATS[b] - PATS[b-1])                (per-bin sums)
  OG[c, n]    = sum_b D[b, c] * [q_n >= g_b]         (staircase gather)
where tab[m, :] = [e^k h2 | e^-k h2] (rank-16, W3 applied after the gather:
out = [W3; -W3]/2 . [e^-q OG_A ; e^q (OG_B - T_B)]).

Masks are {0,2} (DVE/Pool is_ge*2) or +-1 (ACT Sign); both give PATS = 2*PA
- const(b), and the b-difference D cancels the const; the 2x folds into
W3pm = [W3; -W3]/2.

e^{+-k} scales h1 BEFORE layer 2 (relu is positively homogeneous, e^k > 0),
so the layer-2 matmul (lhsT = h1-tile) directly emits transposed h2^T tiles
and its PSUM->SBUF copy doubles as relu.

Sharding: x_target split across 8 cores; m-side replicated. Host prep is
pure relayout/dtype-cast; b2 and b3 must be zero (b1 is folded via a
constant third context feature).
"""

import numpy as np
import ml_dtypes

import concourse.bass as bass
import concourse.tile as tile
from concourse import mybir
from concourse.bass_utils import run_bass_kernel_spmd

N_CORES = 8
M = 8192
N = 8192
N_SH = N // N_CORES    # 1024
H = 16
OUT = 64
NG = 8                 # m-groups stacked across partitions
MJ = M // NG           # 1024
MT = M // 128          # 64 m-tiles
C = 128                # grid boundaries (bin 0 is the -inf sentinel)
GLO, GHI = -4.6, 4.6   # real boundaries g_1..g_127 span [GLO, GHI]
FB = 4                 # masks per buffer tile

# engine pattern for the 64 m-side mask builds (cycled)
MASK_PAT = ("dve", "dve", "act", "dve", "dve", "act", "dve", "act")

F32 = mybir.dt.float32
BF16 = mybir.dt.bfloat16

# ---- blob column offsets (f32 lanes) ----
# bA1a [128, WA1A]: kct | grid16(bcast) | gridcol   (mask inputs, tiny)
KCT0 = 0
GRB0 = KCT0 + MT            # grid bcast bf16: C/2 lanes
GCL0 = GRB0 + C // 2        # grid column f32: 1 lane
WA1A = GCL0 + 1
# bA1b [128, WA1B]: w2bd16 | Mshift
W2B0 = 0                    # w2bd bf16: 64 lanes
MSH0 = W2B0 + 64            # Mshift f32 [128, 128]: 128 lanes
WA1B = MSH0 + 128
# bA2 [128, 1024]: xrep f32
# bA3 [128, WA3]: q16 bcast | identity16
Q16B0 = 0
WA3 = Q16B0 + N_SH // 2
# bB [64, WB]: qb64 f32 | ctxs3 bf16 | w1bd3 bf16 | W3pmA/B bf16
QB0 = 0
CTX0 = QB0 + N_SH           # ctxs3 bf16 [24 rows, MJ]: MJ/2 lanes
W1B0 = CTX0 + MJ // 2       # w1bd3 bf16 [24 rows, 128]: 64 lanes
W3PA0 = W1B0 + 64           # W3pmA bf16 [16 rows, OUT]: 32 lanes
W3PB0 = W3PA0 + OUT // 2    # W3pmB bf16 [32 rows, OUT]: 32 lanes
WB = W3PB0 + OUT // 2


def _build(legalize: bool = True) -> bass.Bass:
    nc = bass.Bass()
    bA1a_d = nc.dram_tensor("bA1a", [128, WA1A], F32, kind="ExternalInput")
    bA1b_d = nc.dram_tensor("bA1b", [128, WA1B], F32, kind="ExternalInput")
    bA2_d = nc.dram_tensor("bA2", [128, MJ], F32, kind="ExternalInput")
    bA3_d = nc.dram_tensor("bA3", [128, WA3], F32, kind="ExternalInput")
    bB_d = nc.dram_tensor("bB", [64, WB], F32, kind="ExternalInput")
    out_d = nc.dram_tensor("out", [OUT, N_SH], F32, kind="ExternalOutput")

    with tile.TileContext(nc) as tc:
        with (
            tc.tile_pool(name="const", bufs=1) as const,
            tc.tile_pool(name="mb_dve", bufs=40) as mb_dve,
            tc.tile_pool(name="mb_act", bufs=24) as mb_act,
                        tc.tile_pool(name="mlpps", bufs=1, space="PSUM") as mlpps,
            tc.tile_pool(name="t2ps", bufs=2, space="PSUM") as t2ps,
            tc.tile_pool(name="hps", bufs=1, space="PSUM") as hps,
            tc.tile_pool(name="ogps", bufs=1, space="PSUM") as ogps,
        ):
            bA1a = const.tile([128, WA1A], F32)
            nc.gpsimd.dma_start(out=bA1a[:], in_=bA1a_d[:, :])
            bA1b = const.tile([128, WA1B], F32)
            nc.gpsimd.dma_start(out=bA1b[:], in_=bA1b_d[:, :])
            bA2 = const.tile([128, MJ], F32)
            nc.sync.dma_start(out=bA2[:], in_=bA2_d[:, :])
            bB = const.tile([64, WB], F32)
            nc.scalar.dma_start(out=bB[:], in_=bB_d[:, :])
            bA3 = const.tile([128, WA3], F32)
            nc.gpsimd.dma_start(out=bA3[:], in_=bA3_d[:, :])

            kct = bA1a[:, KCT0:KCT0 + MT]
            grid16 = bA1a[:, GRB0:GRB0 + C // 2].bitcast(BF16)
            gridcol = bA1a[:, GCL0:GCL0 + 1]
            w2bd16 = bA1b[:, W2B0:W2B0 + 64].bitcast(BF16)
            xrep = bA2[:, :]
            q16b = bA3[:, Q16B0:Q16B0 + N_SH // 2].bitcast(BF16)
            msh = bA1b[:, MSH0:MSH0 + 128]
            qb64 = bB[:, QB0:QB0 + N_SH]
            ctxs3 = bB[0:24, CTX0:CTX0 + MJ // 2].bitcast(BF16)
            w1bd3 = bB[0:24, W1B0:W1B0 + 64].bitcast(BF16)
            w3pmA = bB[0:16, W3PA0:W3PA0 + OUT // 2].bitcast(BF16)
            w3pmB = bB[0:32, W3PB0:W3PB0 + OUT // 2].bitcast(BF16)

            # early ACT consume of bA1a (so later ACT ops carry one wait only)
            a_seen = const.tile([128, 1], F32)
            nc.scalar.copy(a_seen[:], bA1a[:, 0:1])

            # ---- MLP layer 1 (b1 folded via constant third feature)
            ps1 = mlpps.tile([128, MJ], F32, tag="big")
            for cch in range(MJ // 512):
                nc.tensor.matmul(ps1[:, bass.ts(cch, 512)], w1bd3,
                                 ctxs3[:, bass.ts(cch, 512)],
                                 start=True, stop=True)
            # PE pre-consume of bA1b (w2bd16/msh for the t2/shift matmuls)
            scrA = t2ps.tile([128, 128], F32, tag="t2")
            nc.tensor.matmul(scrA[0:1, 0:1], bA1b[0:1, W2B0:W2B0 + 1],
                             bA1b[0:1, W2B0:W2B0 + 1], start=True, stop=True)

            # ---- e^{+-k} scales (applied to h1; relu is pos-homogeneous)
            A1 = const.tile([128, MJ], BF16)
            nc.scalar.activation(A1[:], xrep,
                                 mybir.ActivationFunctionType.Exp, scale=1.0)
            A2 = const.tile([128, MJ], BF16)
            nc.scalar.activation(A2[:], xrep,
                                 mybir.ActivationFunctionType.Exp, scale=-1.0)
            # DVE pre-consume of A1 and A2 (scheduler may order either first)
            a1_seen = const.tile([128, 1], F32)
            nc.vector.tensor_copy(a1_seen[:], A1[:, 0:1])
            a2_seen = const.tile([128, 1], F32)
            nc.vector.tensor_copy(a2_seen[:], A2[:, 0:1])
            h1a = const.tile([128, MJ], BF16)
            nc.vector.scalar_tensor_tensor(
                out=h1a[:], in0=ps1[:], scalar=0.0, in1=A1[:],
                op0=mybir.AluOpType.max, op1=mybir.AluOpType.mult)
            h1b = const.tile([128, MJ], BF16)
            nc.vector.scalar_tensor_tensor(
                out=h1b[:], in0=ps1[:], scalar=0.0, in1=A2[:],
                op0=mybir.AluOpType.max, op1=mybir.AluOpType.mult)

            # negated k columns for ACT sign masks
            nkct = const.tile([128, MT], F32)
            nc.vector.tensor_scalar(out=nkct[:], in0=kct, scalar1=-1.0,
                                    scalar2=None, op0=mybir.AluOpType.mult)

            # ---- layer 2 as transpose: per jt, out [128 m', (g,h)] = h2x^T,
            # interleaved with the m-side mask/hist stream (jt-outer order).
            # tab cols: jt*256 + g*32 + ab*16 + h (hist lhsT slices contiguous)
            tab = const.tile([128, NG * 256], BF16)
            tabv = tab[:].rearrange("p (j g a c) -> p j g a c", j=NG, g=NG, a=2)
            COPY_ENG = ("dve", "act", "dve", "act", "dve", "act", "dve", "act")
            PATS = hps.tile([C, 32], F32, tag="pats")
            nmask = 0
            for jt in range(NG):
                ce = COPY_ENG[jt]
                pt = t2ps.tile([128, 256], F32, tag="t2")
                for ab, h1x in ((0, h1a), (1, h1b)):
                    nc.tensor.matmul(pt[:, ab * 128:(ab + 1) * 128],
                                     h1x[:, bass.ts(jt, 128)], w2bd16,
                                     start=True, stop=True,
                                     skip_group_check=True)
                # relu fused into ONE strided PSUM->SBUF copy; b2 must be 0.
                # dst iterates (g, ab, h); src psum iterates (ab, g, h).
                dst = tabv[:, jt, :, :, :]
                src = pt[:].rearrange("p (a g c) -> p g a c", a=2, g=NG)
                if ce == "act":
                    nc.scalar.activation(dst, src,
                                         mybir.ActivationFunctionType.Relu)
                else:
                    nc.vector.tensor_scalar(out=dst, in0=src,
                                            scalar1=0.0, scalar2=None,
                                            op0=mybir.AluOpType.max)
                # PE pre-consume of tab[jt] so hist matmuls keep only the
                # mask wait (one scr matmul per jt, same copy engine)
                scr = t2ps.tile([128, 256], F32, tag="t2")
                nc.tensor.matmul(scr[0:1, 0:1], tab[0:1, jt * 256:jt * 256 + 1],
                                 tab[0:1, jt * 256 + 16:jt * 256 + 17],
                                 start=True, stop=True, skip_group_check=True)
            allmks = []
            for jt in range(NG):
                for g in range(NG):
                    t = g * NG + jt          # m-tile (chunk) index for kct
                    eng = MASK_PAT[nmask % len(MASK_PAT)]
                    nmask += 1
                    if eng == "act":
                        mk = mb_act.tile([128, C], BF16)
                        nc.scalar.activation(mk[:], grid16,
                                             mybir.ActivationFunctionType.Sign,
                                             bias=nkct[:, t:t + 1])
                    else:
                        mk = mb_dve.tile([128, C], BF16)
                        nc.vector.tensor_scalar(out=mk[:], in0=grid16,
                                        scalar1=kct[:, t:t + 1], scalar2=2.0,
                                        op0=mybir.AluOpType.is_ge,
                                        op1=mybir.AluOpType.mult)
                    allmks.append(mk)
            for jt in range(NG):
                for g in range(NG):
                    rhs = tab[:, jt * 256 + g * 32:jt * 256 + g * 32 + 32]
                    nc.tensor.matmul(PATS[:], allmks[jt * NG + g][:], rhs,
                                     start=(jt == 0 and g == 0),
                                     stop=(jt == NG - 1 and g == NG - 1),
                                     skip_group_check=True)

            # ---- D[b] = PATST[b] - PATST[b-1] via shift matmul (f32)
            patsb = const.tile([C, 32], F32)
            nc.vector.tensor_copy(patsb[:], PATS[:])
            Dps = hps.tile([C, 32], F32, tag="dps")
            nc.tensor.matmul(Dps[:], msh, patsb[:], start=True, stop=True,
                             skip_group_check=True)
            dsb = const.tile([C, 32], BF16)
            nc.vector.tensor_copy(dsb[:], Dps[:])
            ones16 = const.tile([C, 1], BF16)
            nc.vector.memset(ones16[:], 1.0)

            # ---- query staircase
            sq = const.tile([128, N_SH], BF16)
            nc.vector.tensor_scalar(out=sq[:], in0=q16b,
                                    scalar1=gridcol, scalar2=None,
                                    op0=mybir.AluOpType.is_ge)

            # ---- epilogue factors (early: only need qb64/dt16)
            bqm = const.tile([16, N_SH], F32)
            nc.scalar.activation(bqm[:], qb64[0:16, :],
                                 mybir.ActivationFunctionType.Exp, scale=-1.0)
            bqp = const.tile([32, N_SH], F32)
            nc.scalar.activation(bqp[:], qb64[32:64, :],
                                 mybir.ActivationFunctionType.Exp, scale=1.0)
            # totals 2*T = sum_b D16[b] via ones-column gather — MUST use the
            # same bf16 dsb values the main gather sums, so (OG_B - 2T_B)
            # cancels exactly for q above all k (e^q amplifies any mismatch).
            OGT = ogps.tile([32, 1], F32)
            nc.tensor.matmul(OGT[:], dsb[:], ones16[:], start=True, stop=True,
                             skip_group_check=True)
            scolB = const.tile([32, 1], F32)
            nc.vector.tensor_copy(scolB[:], OGT[:])
            # DVE pre-consume of bqm AND bqp (scheduler may order either first)
            bqm_seen = const.tile([16, 1], F32)
            nc.vector.tensor_copy(bqm_seen[:], bqm[:, 0:1])
            bq_seen = const.tile([32, 1], F32)
            nc.vector.tensor_copy(bq_seen[:], bqp[:, 0:1])

            # ---- gather OG then E1/E2, OF, chunked copy+DMA out
            OG = mlpps.tile([32, N_SH], F32, tag="big")
            OF = mlpps.tile([OUT, N_SH], F32, tag="big")
            ecatA = const.tile([16, N_SH], BF16)
            ecatB = const.tile([32, N_SH], BF16)
            for u in range(N_SH // 512):
                cs = bass.ts(u, 512)
                nc.tensor.matmul(OG[:, cs], dsb[:], sq[:, cs],
                                 start=True, stop=True, skip_group_check=True)
            for u in range(N_SH // 512):
                cs = bass.ts(u, 512)
                nc.vector.tensor_tensor(out=ecatA[:, cs], in0=OG[0:16, cs],
                                        in1=bqm[:, cs], op=mybir.AluOpType.mult)
                nc.vector.scalar_tensor_tensor(
                    out=ecatB[:, cs], in0=OG[0:32, cs],
                    scalar=scolB[:], in1=bqp[:, cs],
                    op0=mybir.AluOpType.subtract, op1=mybir.AluOpType.mult)
                nc.tensor.matmul(OF[:, cs], w3pmA, ecatA[:, cs], start=True,
                                 stop=False, skip_group_check=True)
                nc.tensor.matmul(OF[:, cs], w3pmB, ecatB[:, cs], start=False,
                                 stop=True, skip_group_check=True)
                osb = const.tile([OUT, 512], F32, tag=f"osb{u}")
                nc.vector.tensor_copy(osb[:], OF[:, cs])
                (nc.sync if u == 0 else nc.scalar).dma_start(
                    out=out_d[:, u * 512:(u + 1) * 512], in_=osb[:])

    if legalize:
        _fix_tsp_waits(nc)
    return nc


def _fix_tsp_waits(nc: bass.Bass) -> None:
    """Walrus accepts at most ONE sync-wait per compute instruction (and few
    on the tail drain). Same-engine self-waits are redundant — every engine
    completes its queue strictly in order — so drop them; the tail drain
    keeps only the output-DMA wait (the dag funnels through it)."""
    budget = {"InstTensorScalarPtr": 1, "InstMatmult": 1, "InstTensorCopy": 1,
              "InstMemset": 1, "InstActivation": 1, "InstTensorTensor": 1,
              "InstScalarTensorTensor": 1}
    eng_prefix = {"DVE": "DVE_", "Activation": "Activation_", "PE": "PE_",
                  "SP": "SP_", "Pool": "Pool_"}
    blocks = nc.m.functions[0].blocks
    out_dma_sems: set[str] = set()
    for b in blocks:
        for inst in b.instructions:
            if type(inst).__name__ == "InstDMACopy" and inst.sync_info:
                out_dma_sems = {u.ant_name for u in inst.sync_info.on_update
                                if u.ant_name}
    for b in blocks:
        for inst in b.instructions:
            tname = type(inst).__name__
            si = inst.sync_info
            if si is None:
                continue
            if tname == "InstDrain" and len(si.on_wait) > 1:
                kept = [w for w in si.on_wait if w.ant_name in out_dma_sems]
                if len(kept) != 1:
                    raise RuntimeError(f"tail drain {inst.name}: waits "
                                       f"{[(w.ant_name, w.wait_value) for w in si.on_wait]}")
                si.on_wait = kept
                inst.sync_info = si
                continue
            lim = budget.get(tname)
            if lim is None or len(si.on_wait) <= lim:
                continue
            eng = str(inst.engine).split(".")[-1]
            pfx = eng_prefix.get(eng, "\x00")
            kept = [w for w in si.on_wait
                    if not (w.ant_name or "").startswith(pfx)]
            if len(kept) > lim:
                raise RuntimeError(
                    f"{inst.name} ({tname}, {eng}): "
                    f"{[(w.ant_name, w.wait_value) for w in si.on_wait]}")
            si.on_wait = kept
            inst.sync_info = si


def _bf(a):
    return np.ascontiguousarray(
        np.asarray(a, dtype=np.float32).astype(ml_dtypes.bfloat16)
    ).view(np.float32)


def _prep_maps(inputs: dict) -> list[dict]:
    xc = np.ascontiguousarray(inputs["x_context"], dtype=np.float32).reshape(M)
    yc = np.ascontiguousarray(inputs["y_context"], dtype=np.float32).reshape(M)
    xt = np.ascontiguousarray(inputs["x_target"], dtype=np.float32).reshape(N)
    W1 = np.asarray(inputs["W1"], dtype=np.float32)
    b1 = np.asarray(inputs["b1"], dtype=np.float32)
    W2 = np.asarray(inputs["W2"], dtype=np.float32)
    W3 = np.asarray(inputs["W3"], dtype=np.float32)

    gridf = np.empty(C, np.float32)
    gridf[0] = -1e30
    gridf[1:] = GLO + (GHI - GLO) * np.arange(C - 1, dtype=np.float64) / (C - 2)
    grid = gridf.astype(ml_dtypes.bfloat16).astype(np.float32)  # device-consistent

    kct = np.ascontiguousarray(xc.reshape(MT, 128).T)            # [128, MT]
    xrep = np.repeat(xc.reshape(NG, MJ), H, axis=0)              # [128, MJ]

    ctxs3 = np.zeros((24, MJ), dtype=np.float32)
    w1bd3 = np.zeros((24, 128), dtype=np.float32)
    for g in range(NG):
        ctxs3[3 * g + 0] = xc.reshape(NG, MJ)[g]
        ctxs3[3 * g + 1] = yc.reshape(NG, MJ)[g]
        ctxs3[3 * g + 2] = 1.0
        w1bd3[3 * g + 0, H * g:H * (g + 1)] = W1[0]
        w1bd3[3 * g + 1, H * g:H * (g + 1)] = W1[1]
        w1bd3[3 * g + 2, H * g:H * (g + 1)] = b1
    w2bd = np.zeros((128, 128), dtype=np.float32)
    for g in range(NG):
        w2bd[H * g:H * (g + 1), H * g:H * (g + 1)] = W2
    w3pmA = W3 * 0.5                                             # [16, OUT]
    w3pmB = np.concatenate([np.zeros((16, OUT), np.float32), -W3 * 0.5], 0)

    msh = np.zeros((C, C), dtype=np.float32)
    for b in range(1, C):
        msh[b, b] = 1.0
        msh[b - 1, b] = -1.0
    bA1a = np.zeros((128, WA1A), dtype=np.float32)
    bA1a[:, KCT0:KCT0 + MT] = kct
    bA1a[:, GRB0:GRB0 + C // 2] = np.broadcast_to(_bf(grid)[None, :], (128, C // 2))
    bA1a[:, GCL0] = grid
    bA1b = np.zeros((128, WA1B), dtype=np.float32)
    bA1b[:, W2B0:W2B0 + 64] = _bf(w2bd)
    bA1b[:, MSH0:MSH0 + 128] = msh
    bA2 = np.ascontiguousarray(xrep)
    bA3 = np.zeros((128, WA3), dtype=np.float32)
    bB0 = np.zeros((64, WB), dtype=np.float32)
    bB0[0:24, CTX0:CTX0 + MJ // 2] = _bf(ctxs3)
    bB0[0:24, W1B0:W1B0 + 64] = _bf(w1bd3)
    bB0[0:16, W3PA0:W3PA0 + OUT // 2] = _bf(w3pmA)
    bB0[0:32, W3PB0:W3PB0 + OUT // 2] = _bf(w3pmB)

    maps = []
    for cid in range(N_CORES):
        q = xt[cid * N_SH:(cid + 1) * N_SH]
        a3 = bA3.copy()
        a3[:, Q16B0:Q16B0 + N_SH // 2] = np.broadcast_to(
            _bf(q)[None, :], (128, N_SH // 2))
        bb = bB0.copy()
        bb[:, QB0:QB0 + N_SH] = q[None, :]
        maps.append({"bA1a": bA1a, "bA1b": bA1b, "bA2": bA2, "bA3": a3, "bB": bb})
    return maps


def _unshard(results: list[dict], b2, b3) -> np.ndarray:
    if np.any(np.asarray(b2)) or np.any(np.asarray(b3)):
        raise RuntimeError("nonzero b2/b3 unsupported by this kernel")
    out = np.empty((N, OUT), dtype=np.float32)
    for cid in range(N_CORES):
        out[cid * N_SH:(cid + 1) * N_SH] = results[cid]["out"].T
    return out


def run(inputs: dict, **spmd_kwargs):
    nc = _build()
    in_maps = _prep_maps(inputs)
    res = run_bass_kernel_spmd(nc, in_maps, list(range(N_CORES)), **spmd_kwargs)
    return _unshard(res.results, inputs["b2"], inputs["b3"]), res


def kernel(**inputs) -> np.ndarray:
    out, _ = run(inputs)
    return out
